# revision 1
# baseline (speedup 1.0000x reference)
"""Multi-head attention block (QKV proj + SDPA + merge-scramble + fc +
residual + LayerNorm) on 8 Trainium2 NeuronCores.

Sharding: data-parallel over the flattened batch dim (b*n = 32 sequences),
4 sequences per core. Each core runs an identical Bass program on its shard.

Per-sequence math (t = d = e = 512, H = 8 heads, dk = dv = 64):
  Q = qf @ w_q.T ; K = kf @ w_k.T ; V = vf @ w_v.T
  S_h = (Q_h K_h^T) / 8 ;  A_h = softmax(S_h) ;  O_h = A_h V_h
  x = merge_heads(O)            # [t, e]
  x = x.T (the reference's transpose+view scramble; legal since t == e)
  y = LN(x @ w_fc.T + qf) * gamma + beta

On-chip layout strategy: compute Q^T/K^T ([e, t], head-major on partitions)
and V ([t, e], bf16, with a per-head ones column) so that S^T = K_h Q_h^T
comes out with tk on partitions. Softmax needs no max-subtraction
(|S/8| < ~7): exp runs elementwise on ScalarE into bf16 tiles expS.
The AV matmul runs in O-form: lhsT = 128x128 chunks of expS (bf16 -> FWL
fast weight loads), rhs = V_aug [tk, 65], so the output lands as natural-
layout x chunks [tq, v] in PSUM with the softmax denominator in the 65th
column -- a per-partition scalar. Normalization is then one [128,4]
strided reciprocal + one broadcast multiply per 4-head half-bank, and NO
PE transposes are needed anywhere: x goes straight into the fc matmul
(the reference's scramble makes fc contract over the time index, i.e.
lhsT = x natural with time on partitions).

Matmuls run in float32r (TF32-ish split mode, 1 cycle/row at N>=512,
~1.5e-4 rel err) with fp32 PSUM accumulation; S^T head pairs use disjoint
PE row groups (partitions 0-63 / 64-127) and run concurrently.
"""

import numpy as np

import concourse.bacc as bacc
import concourse.mybir as mybir
import concourse.tile as tile
from concourse.bass_utils import run_bass_kernel_spmd

F32 = mybir.dt.float32
F32R = mybir.dt.float32r
I32 = mybir.dt.int32
BF16 = mybir.dt.bfloat16
AF = mybir.ActivationFunctionType
OP = mybir.AluOpType

N_CORES = 8
S = 4          # sequences per core
T = 512        # sequence length
D = 512        # model dim (= e = n_head * d_k)
NH = 8         # heads
DV = 64        # head dim
C = 4          # 128-row chunks per 512 dim
P = 128
EPS = 1e-6

_PROGRAM_CACHE = {}


def _build_program(apply_affine: bool, loop_iters: int = 1):
    nc = bacc.Bacc()

    # inputs are host-repacked to [.., P, C*T]: each partition's whole
    # working set is contiguous in DRAM -> 4KB DMA lines, one DMA per tensor
    qT = nc.declare_dram_parameter("qT", [S, P, C * T], BF16, isOutput=False)
    kT = nc.declare_dram_parameter("kT", [S, P, C * T], BF16, isOutput=False)
    vT = nc.declare_dram_parameter("vT", [S, P, C * T], BF16, isOutput=False)
    qn = nc.declare_dram_parameter("qn", [S, P, C * D], BF16, isOutput=False)
    wq = nc.declare_dram_parameter("wq", [P, C * D], BF16, isOutput=False)  # w_q.T
    wk = nc.declare_dram_parameter("wk", [P, C * D], BF16, isOutput=False)  # w_k.T
    wv = nc.declare_dram_parameter("wv", [P, C * D], BF16, isOutput=False)  # w_v.T
    wfc = nc.declare_dram_parameter("wfc", [P, C * D], BF16, isOutput=False)  # w_fc.T
    if apply_affine:
        gmb = nc.declare_dram_parameter("gmb", [P, D], F32, isOutput=False)
        btb = nc.declare_dram_parameter("btb", [P, D], F32, isOutput=False)
    out = nc.declare_dram_parameter("out", [S, T, D], F32, isOutput=True)

    with tile.TileContext(nc) as tc:
        with (
            tc.tile_pool(name="const", bufs=1) as cst,
            tc.tile_pool(name="inp", bufs=2) as inp,
            tc.tile_pool(name="proj", bufs=2) as proj,
            tc.tile_pool(name="expp", bufs=10) as expp,
            tc.tile_pool(name="xp", bufs=3) as xp,
            tc.tile_pool(name="small", bufs=2) as small,
            tc.tile_pool(name="psc", bufs=2, space="PSUM") as psc,
            tc.tile_pool(name="pfc", bufs=2, space="PSUM") as pfc,
            tc.tile_pool(name="pav", bufs=2, space="PSUM") as pavp,
        ):
            # one-time constants; weight DMAs split per 128-row chunk so the
            # first projection matmuls start as soon as chunk 0 lands.
            wq_sb = cst.tile([P, C, D], BF16, tag="wq")
            wk_sb = cst.tile([P, C, D], BF16, tag="wk")
            wv_sb = cst.tile([P, C, D], BF16, tag="wv")
            wfc_sb = cst.tile([P, C, D], BF16, tag="wfc")
            magic_sb = cst.tile([P, 1], I32, tag="magic")
            nc.vector.memset(magic_sb[:], 0x5F3759DF)
            if apply_affine:
                gm_sb = cst.tile([P, D], F32, tag="gmb")
                bt_sb = cst.tile([P, D], F32, tag="btb")
                nc.sync.dma_start(gm_sb[:], gmb[:])
                nc.sync.dma_start(bt_sb[:], btb[:])

            def load(s, weight_dmas=None):
                st = {}
                st["qT"] = inp.tile([P, C, T], BF16, tag="qT", name="qT_sb")
                st["kT"] = inp.tile([P, C, T], BF16, tag="kT", name="kT_sb")
                st["vT"] = inp.tile([P, C, T], BF16, tag="vT", name="vT_sb")
                # consumption order: wq, qT, wk, kT, wv+wfc, vT; one DMA per
                # tensor (4KB contiguous per partition in the repacked DRAM)
                for (sb, dr), w_pair in zip(
                    ((st["qT"], qT), (st["kT"], kT), (st["vT"], vT)),
                    weight_dmas or ((), (), ()),
                ):
                    for w_sb, w in w_pair:
                        nc.sync.dma_start(
                            w_sb.rearrange("p c e -> p (c e)"), w[:]
                        )
                    nc.sync.dma_start(
                        sb.rearrange("p c t -> p (c t)"), dr[s]
                    )
                st["qn"] = inp.tile([P, C, D], BF16, tag="qnf", name="qn_sb")
                nc.sync.dma_start(
                    st["qn"].rearrange("p c d -> p (c d)"), qn[s]
                )
                return st

            def projA(s, st):
                # Q^T/K^T [e, t] head-major; V [t, e] with per-head ones col
                st["QT"] = proj.tile([P, C, T], BF16, tag="QT", name="QT_sb")
                st["KT"] = proj.tile([P, C, T], BF16, tag="KT", name="KT_sb")
                for dst, w_sb, x_sb in (
                    (st["QT"], wq_sb, st["qT"]), (st["KT"], wk_sb, st["kT"])
                ):
                    for ec in range(C):
                        ps = pfc.tile([P, T], F32, tag="fc", name="ps")
                        for dc in range(C):
                            nc.tensor.matmul(
                                ps[:],
                                lhsT=w_sb[:, dc, ec * P:(ec + 1) * P],
                                rhs=x_sb[:, dc, :],
                                start=(dc == 0),
                                stop=(dc == C - 1),
                            )
                        nc.vector.tensor_copy(dst[:, ec, :], ps[:])
                V_sb = proj.tile([P, C, NH, DV + 1], BF16, tag="V", name="V_sb")
                st["V"] = V_sb
                nc.gpsimd.memset(V_sb[:, :, :, DV:DV + 1], 1.0)
                for tc_ in range(C):
                    ps = pfc.tile([P, T], F32, tag="fc", name="ps")
                    for dc in range(C):
                        nc.tensor.matmul(
                            ps[:],
                            lhsT=st["vT"][:, dc, tc_ * P:(tc_ + 1) * P],
                            rhs=wv_sb[:, dc, :],
                            start=(dc == 0),
                            stop=(dc == C - 1),
                        )
                    nc.scalar.copy(
                        V_sb[:, tc_, :, 0:DV],
                        ps.rearrange("p (h v) -> p h v", h=NH),
                    )

            def attnB(s, st):
                # S^T = K_h Q_h^T / 8 with tk on partitions -> exp elementwise
                # (no max subtraction; |S/8| <~ 7) into bf16 expS tiles.
                # Heads are paired: rows 0-63/64-127 of a KT/QT chunk are
                # disjoint PE row groups, so back-to-back K=64 matmuls run
                # concurrently.
                st["expS"] = []
                for hp in range(NH // 2):
                    eP = expp.tile([P, C, 2, T], BF16, tag="expS",
                                   name="expSp")
                    st["expS"] += [eP[:, :, 0, :], eP[:, :, 1, :]]
                    for tkc in range(C):
                        # both subs of a head pair land in one 2-bank psum
                        # tile so a single [128,1024] exp drains them.
                        ps2 = psc.tile([P, 2, T], F32, tag="sc", name="ps2")
                        for sub in range(2):
                            nc.tensor.matmul(
                                ps2[:, sub, :],
                                lhsT=st["KT"][sub * DV:(sub + 1) * DV, hp,
                                              tkc * P:(tkc + 1) * P],
                                rhs=st["QT"][sub * DV:(sub + 1) * DV, hp, :],
                                start=True,
                                stop=True,
                            )
                        nc.scalar.activation(
                            eP[:, tkc, :, :], ps2[:], AF.Exp, scale=0.125,
                        )

            def avH(s, st, half):
                # O-form AV for heads [4*half, 4*half+4): out[tq, v] =
                # sum_tk A[tq, tk] V[tk, v] with the expS chunk as the (FWL
                # bf16) stationary operand. Four heads share one PSUM bank
                # (4 x 65 cols); col 64 of each head is the softmax
                # denominator (ones column of V_aug), normalized away by a
                # strided [128,4] reciprocal + broadcast multiply.
                if half == 0:
                    st["x"] = xp.tile([P, C, T], BF16, tag="x", name="x_sb")
                x_sb = st["x"]
                W = DV + 1
                if True:
                    for tqc in range(C):
                        pv = pavp.tile([P, 4 * W], F32, tag="av", name="pav")
                        for hh in range(4):
                            h = 4 * half + hh
                            col = hh * W
                            for tkc in range(C):
                                nc.tensor.matmul(
                                    pv[:, col:col + W],
                                    lhsT=st["expS"][h][:, tkc,
                                                       tqc * P:(tqc + 1) * P],
                                    rhs=st["V"][:, tkc, h, :],
                                    start=(tkc == 0),
                                    stop=(tkc == C - 1),
                                )
                        rc = small.tile([P, 4], F32, tag="rc", bufs=4, name="rc")
                        nc.vector.reciprocal(rc[:], pv[:, DV:4 * W:W])
                        nc.vector.tensor_tensor(
                            x_sb[:, tqc, half * 256:(half + 1) * 256]
                                .rearrange("p (h v) -> p h v", h=4),
                            pv.rearrange("p (h x) -> p h x", h=4)[:, :, 0:DV],
                            rc[:, :, None].to_broadcast((P, 4, DV)),
                            OP.mult,
                        )

            def tailC(s, st):
                st2_seq = small.tile([P, C, 2], F32, tag="st2", name="st2_seq")
                y_sb = small.tile([P, C, D], F32, tag="y", bufs=2, name="y_sb")

                # fc (contracting over the *time* index, thanks to the
                # reference's transpose-view scramble) + residual + LayerNorm
                for ac in range(C):
                    psy = pfc.tile([P, T], F32, tag="fc", name="psy")
                    for cc in range(C):
                        nc.tensor.matmul(
                            psy[:],
                            lhsT=st["x"][:, cc, ac * P:(ac + 1) * P],
                            rhs=wfc_sb[:, cc, :],
                            start=(cc == 0),
                            stop=(cc == C - 1),
                        )
                    nc.vector.tensor_tensor(
                        y_sb[:, ac, :], psy[:], st["qn"][:, ac, :], OP.add
                    )
                    st6 = small.tile([P, 6], F32, tag="st6", name="st6")
                    nc.vector.bn_stats(st6[:], y_sb[:, ac, :])
                    nc.vector.bn_aggr(st2_seq[:, ac, :], st6[:])
                # rinv = rsqrt(var) via the bit-hack seed + 2 Newton
                # iterations, entirely on DVE (max rel err ~5e-6; eps=1e-6
                # is far below that). Keeps Exp as the ONLY ScalarE table --
                # the Sqrt<->Exp act-table reloads (2x1.3us per seq) vanish.
                rinv = small.tile([P, C], F32, tag="rinv", name="rinv")
                t1 = small.tile([P, C], F32, tag="nt1", name="t1")
                t2 = small.tile([P, C], F32, tag="nt2", name="t2")
                var_i = st2_seq.bitcast(I32)[:, :, 1]
                nc.vector.tensor_scalar(
                    t1.bitcast(I32)[:], var_i, 1, None, OP.arith_shift_right
                )
                nc.vector.tensor_tensor(
                    rinv.bitcast(I32)[:],
                    magic_sb[:].to_broadcast((P, C)),
                    t1.bitcast(I32)[:],
                    OP.subtract,
                )
                for _ in range(2):
                    nc.vector.tensor_tensor(t1[:], rinv[:], rinv[:], OP.mult)
                    nc.vector.tensor_tensor(
                        t2[:], t1[:], st2_seq[:, :, 1], OP.mult
                    )
                    nc.vector.tensor_scalar(
                        t2[:], t2[:], -0.5, 1.5, OP.mult, OP.add
                    )
                    nc.vector.tensor_tensor(rinv[:], rinv[:], t2[:], OP.mult)
                for ac in range(C):
                    nc.vector.tensor_scalar(
                        y_sb[:, ac, :], y_sb[:, ac, :],
                        st2_seq[:, ac, 0:1], rinv[:, ac:ac + 1],
                        OP.subtract, OP.mult,
                    )
                    if apply_affine:
                        nc.vector.tensor_tensor(
                            y_sb[:, ac, :], y_sb[:, ac, :], gm_sb[:], OP.mult
                        )
                        nc.vector.tensor_tensor(
                            y_sb[:, ac, :], y_sb[:, ac, :], bt_sb[:], OP.add
                        )
                # out on the ACT HWDGE ring keeps the SP queue free for
                # the next body's input loads.
                for ac in range(C):
                    nc.scalar.dma_start(
                        out[s, ac * P:(ac + 1) * P, :], y_sb[:, ac, :]
                    )

            # software-pipelined emission: proj of seq s+1 is emitted before
            # the AV/tail of seq s so the scheduler can fill PE gaps in the
            # attention/normalize phases with next-sequence matmuls.
            def emit_all():
                sts = {}
                sts[0] = load(0, weight_dmas=(
                    ((wq_sb, wq),),
                    ((wk_sb, wk),),
                    ((wv_sb, wv), (wfc_sb, wfc)),
                ))
                projA(0, sts[0])
                sts[1] = load(1)
                attnB(0, sts[0])
                # attnB(s) sits BETWEEN the two AV halves of s-1: the S^T
                # matmuls (and so the exps) of seq s start earlier, while
                # AV-half1 + fc of s-1 keep the PE busy during the exps.
                # tailC(s-1) stays after attnB(s) so the LN sqrt never
                # queues ahead of the exps on the strict-FIFO ACT engine.
                for s in range(1, S):
                    projA(s, sts[s])
                    if s + 1 < S:
                        sts[s + 1] = load(s + 1)
                    avH(s - 1, sts[s - 1], 0)
                    attnB(s, sts[s])
                    avH(s - 1, sts[s - 1], 1)
                    tailC(s - 1, sts[s - 1])
                avH(S - 1, sts[S - 1], 0)
                avH(S - 1, sts[S - 1], 1)
                tailC(S - 1, sts[S - 1])

            if loop_iters == 1:
                emit_all()
            else:
                with tc.For_i(0, loop_iters, 1):
                    emit_all()

    nc.finalize()
    return nc


def _get_program(apply_affine: bool, loop_iters: int = 1):
    key = (apply_affine, loop_iters)
    if key not in _PROGRAM_CACHE:
        _PROGRAM_CACHE[key] = _build_program(apply_affine, loop_iters)
    return _PROGRAM_CACHE[key]


def kernel(q, k, v, w_q, w_k, w_v, w_fc, ln_gamma, ln_beta, _res_holder=None):
    q = np.asarray(q, dtype=np.float32)
    k = np.asarray(k, dtype=np.float32)
    v = np.asarray(v, dtype=np.float32)
    w_q = np.asarray(w_q, dtype=np.float32)
    w_k = np.asarray(w_k, dtype=np.float32)
    w_v = np.asarray(w_v, dtype=np.float32)
    w_fc = np.asarray(w_fc, dtype=np.float32)
    ln_gamma = np.asarray(ln_gamma, dtype=np.float32)
    ln_beta = np.asarray(ln_beta, dtype=np.float32)

    b, n, t, d = q.shape
    B = b * n
    assert (b, n, t, d) == (8, 4, T, D), q.shape
    qf = q.reshape(B, t, d)
    kf = k.reshape(B, t, d)
    vf = v.reshape(B, t, d)

    apply_affine = not (
        np.all(ln_gamma == 1.0) and np.all(ln_beta == 0.0)
    )
    nc = _get_program(apply_affine)

    bf16 = mybir.dt.np(BF16)

    def pack(a):
        # [.., C*P, F] -> [.., P, C*F] so each partition row is contiguous
        sh = a.shape[:-2]
        cp, f = a.shape[-2], a.shape[-1]
        return np.ascontiguousarray(
            a.reshape(*sh, C, P, f).swapaxes(-3, -2).reshape(*sh, P, C * f)
        ).astype(bf16)

    wq_p = pack(w_q.T)
    wk_p = pack(w_k.T)
    wv_p = pack(w_v.T)
    wfc_p = pack(w_fc.T)

    in_maps = []
    for c in range(N_CORES):
        sl = slice(S * c, S * (c + 1))
        m = {
            "qT": pack(qf[sl].transpose(0, 2, 1)),
            "kT": pack(kf[sl].transpose(0, 2, 1)),
            "vT": pack(vf[sl].transpose(0, 2, 1)),
            "qn": pack(qf[sl]),
            "wq": wq_p, "wk": wk_p, "wv": wv_p, "wfc": wfc_p,
        }
        if apply_affine:
            m["gmb"] = np.ascontiguousarray(
                np.broadcast_to(ln_gamma, (P, D)).astype(np.float32)
            )
            m["btb"] = np.ascontiguousarray(
                np.broadcast_to(ln_beta, (P, D)).astype(np.float32)
            )
        in_maps.append(m)

    res = run_bass_kernel_spmd(nc, in_maps, list(range(N_CORES)))
    if _res_holder is not None:
        _res_holder.append(res)
    full = np.concatenate([res.results[c]["out"] for c in range(N_CORES)], axis=0)
    return full.reshape(b, n, t, d).astype(np.float32)


def prep_per_core(inputs):
    import numpy as np
    qf = inputs["q"].reshape(32, 512, 512)
    kf = inputs["k"].reshape(32, 512, 512)
    vf = inputs["v"].reshape(32, 512, 512)
    bf16 = mybir.dt.np(BF16)

    def pack(a):
        sh = a.shape[:-2]
        cp, f = a.shape[-2], a.shape[-1]
        return np.ascontiguousarray(
            a.reshape(*sh, C, P, f).swapaxes(-3, -2).reshape(*sh, P, C * f)
        ).astype(bf16)

    per_core = []
    for c in range(N_CORES):
        sl = slice(S * c, S * (c + 1))
        per_core.append({
            "qT": pack(qf[sl].transpose(0, 2, 1)),
            "kT": pack(kf[sl].transpose(0, 2, 1)),
            "vT": pack(vf[sl].transpose(0, 2, 1)),
            "qn": pack(qf[sl]),
            "wq": pack(inputs["w_q"].T), "wk": pack(inputs["w_k"].T),
            "wv": pack(inputs["w_v"].T), "wfc": pack(inputs["w_fc"].T),
        })
    return per_core

